# revision 39
# baseline (speedup 1.0000x reference)
"""Trainium2 Bass kernel for nn_MfdFC (weighted Frechet mean on S^7).

Self-contained: kernel(x, w) -> [8,1024,32,8] float32.

Math per point b, out-channel o, 3 fixed iterations:
  c[o,i] = <a_o, x_i>;  t1 = min(c-1, -1e-3);  u = t1*(t1+2) = c^2-1
  rs = 1/sqrt(|u|);  tn = t1*rs = -tan(theta/2);  at = arctan(tn) = -theta/2
  fw = at * (rs * (-2*wn)) = wn * theta/sin(theta)
  gr[o,:] = sum_i fw[o,i] x_i;  gs = <a_o,gr>;  n2 = |gr|^2 - gs^2
  anew = (cos(n) - sinc(n)*gs)*a + sinc(n)*gr   (Taylor sinc/cos in n2)

All matmuls in fp16 (fp32 psum).  Per core: 1024 points = 4 megagroups
(mg) x (sg,g,j,p) each in [0,4).  Layouts:
  XC   [(g,i), (s,j,p,d)]   grad-mm moving / it0 elementwise
  X4BD [(p,gd), (s,j,(g,i))] inner-mm stationary, block-diag over g
  X0C  [(g,i), (s,j,p,d)]   x0 broadcast over i (it0)
  AALL [(p,o), (mg,s,j,g,d)] current a (grad-mm/exp layout)
  TT   [(p,g,d), (s,j,o)]   transpose of AALL mg-slice = inner-mm moving
  C    [(g,i), (s,j,p,o)]   inner products (psum -> sbuf f32 via DMA)
  GR   [(p,o), (s,j,g,d)]   psum grad per mg
Inner mm: stationary [32,128] block-diag x at tile row 32p -> 32 moving
rows (o) produce 128x32 outputs; 4x fewer PE rows than the dense form.
"""

import numpy as np

B, L, CIN, COUT, D = 8, 1024, 32, 32, 8
NCORES = 8
PTS_PER_CORE = (B * L) // NCORES  # 1024
N_MG = 4
N_SG = 4
N_ITER = 3
DELTA = np.float32(1e-3)

_CACHE = {}


# --------------------------------------------------------------------------
# host-side layout packing
# --------------------------------------------------------------------------
def _host_prep(x, w):
    xs = np.ascontiguousarray(x, dtype=np.float32).reshape(B * L, CIN, D)
    wf = np.asarray(w, dtype=np.float32)
    wn = np.exp(wf)
    wn = wn / wn.sum(axis=0, keepdims=True)          # [CIN(i), COUT(o)]

    i_idx = np.arange(128) % 32
    # WNF2 [(g,i), (s,j,p,o)] = -2*wn[i,o]
    wnf2 = np.tile((-2.0 * wn)[i_idx][:, None, :], (1, 64, 1)).reshape(
        128, 2048).astype(np.float16)
    # WN4 [(g,i), o] = -wn[i,o]
    wn4 = np.ascontiguousarray((-wn)[i_idx]).astype(np.float16)  # [128, 32]

    x16 = xs.astype(np.float16)
    per_core = []
    for cco in range(NCORES):
        pts = x16[cco * PTS_PER_CORE:(cco + 1) * PTS_PER_CORE]  # [1024,32,8]
        # pt = mg*256 + sg*64 + g*16 + j*4 + p
        v = pts.reshape(N_MG, N_SG, 4, 4, 4, CIN, D)  # [mg,s,g,j,p,i,d]

        # XC [mg, (g,i), (s,j,p,d)]
        xc = np.ascontiguousarray(
            v.transpose(0, 2, 5, 1, 3, 4, 6)  # mg,g,i,s,j,p,d
        ).reshape(N_MG, 128, 512)

        # X4BD [mg, (p,g',d), (s,j,g,i)] block diag over g
        x4bd = np.zeros((N_MG, 4, 4, 8, N_SG, 4, 4, 32), np.float16)
        vt = v.transpose(0, 4, 2, 6, 1, 3, 5)  # mg,p,g,d,s,j,i
        for g in range(4):
            x4bd[:, :, g, :, :, :, g, :] = vt[:, :, g]
        x4bd = x4bd.reshape(N_MG, 128, 2048)

        x0 = v[:, :, :, :, :, 0, :]  # [mg,s,g,j,p,d]
        # X0C [mg, (g,i), (s,j,p,d)] = x0 broadcast over i
        x0t = x0.transpose(0, 2, 1, 3, 4, 5)  # mg,g,s,j,p,d
        x0c = np.ascontiguousarray(np.broadcast_to(
            x0t[:, :, None], (N_MG, 4, 32, N_SG, 4, 4, D))
        ).reshape(N_MG, 128, 512)

        # AALL0 [(p,o), (mg,s,j,g,d)] = x0 broadcast over o
        a0 = x0.transpose(0, 4, 1, 3, 2, 5)  # mg,p,s,j,g,d
        aall0 = np.ascontiguousarray(np.broadcast_to(
            a0[:, :, None], (N_MG, 4, 32, N_SG, 4, 4, D))
        ).transpose(1, 2, 0, 3, 4, 5, 6).reshape(128, 2048)
        aall0 = np.ascontiguousarray(aall0)

        per_core.append({"xc": xc, "x4bd": x4bd, "x0c": x0c,
                         "aall0": aall0, "wnf2": wnf2, "wn4": wn4})
    return per_core


def _host_unpack(outs):
    """outs: list of 8 arrays [128, 2048] f32 [(p,o),(mg,s,j,g,d)]."""
    res = np.empty((B * L, COUT, D), np.float32)
    for cco, oe in enumerate(outs):
        vv = oe.astype(np.float32).reshape(4, 32, N_MG, N_SG, 4, 4, D)
        vv = vv.transpose(2, 3, 5, 4, 0, 1, 6)       # mg,s,g,j,p,o,d
        res[cco * PTS_PER_CORE:(cco + 1) * PTS_PER_CORE] = vv.reshape(
            PTS_PER_CORE, COUT, D)
    return res.reshape(B, L, COUT, D)


# --------------------------------------------------------------------------
# bass program
# --------------------------------------------------------------------------
def build_bass(strip=True, dma_extract=True):
    import concourse.bass as bass
    import concourse.mybir as mybir
    from concourse.tile import TileContext
    from concourse.tile_rust import add_dep_helper

    f32 = mybir.dt.float32
    f16 = mybir.dt.float16
    AF = mybir.ActivationFunctionType
    OP = mybir.AluOpType

    nc = bass.Bass()

    def act(out, in_, func, scale=1.0, bias=0.0):
        eng = nc.scalar
        if func in (AF.Copy, AF.Reciprocal):
            bias_arg = mybir.ImmediateValue(dtype=mybir.dt.float32,
                                            value=float(bias))
        else:
            bias_arg = eng.lower_ap(nc.const_aps.scalar_like(float(bias),
                                                             in_))
        return eng.add_instruction(mybir.InstActivation(
            name=nc.get_next_instruction_name(), func=func,
            ins=[eng.lower_ap(in_), bias_arg,
                 mybir.ImmediateValue(dtype=mybir.dt.float32,
                                      value=float(scale)),
                 mybir.ImmediateValue(dtype=mybir.dt.float32, value=0.0)],
            outs=[eng.lower_ap(out)]))

    def eng_fence(eng, deps):
        last = None
        for di in deps:
            if di is None:
                continue
            n = eng.nop(nofuse=True, hint="fence")
            add_dep_helper(n.ins, di.ins, reason="fence")
            last = n
        return last

    def pe_fence(deps):
        return eng_fence(nc.tensor, deps)

    xc_d = nc.dram_tensor("xc", [N_MG, 128, 512], f16, kind="ExternalInput")
    x4_d = nc.dram_tensor("x4bd", [N_MG, 128, 2048], f16,
                          kind="ExternalInput")
    x0c_d = nc.dram_tensor("x0c", [N_MG, 128, 512], f16,
                           kind="ExternalInput")
    aall0_d = nc.dram_tensor("aall0", [128, 2048], f16, kind="ExternalInput")
    wnf2_d = nc.dram_tensor("wnf2", [128, 2048], f16, kind="ExternalInput")
    wn4_d = nc.dram_tensor("wn4", [128, 32], f16, kind="ExternalInput")
    out_d = nc.dram_tensor("outa", [128, 2048], f16, kind="ExternalOutput")

    _alldmas = []

    with TileContext(nc) as tc:
        with (
            tc.tile_pool(name="const", bufs=1) as constp,
            tc.tile_pool(name="io", bufs=1) as iop,
            tc.tile_pool(name="big", bufs=2) as bigp,
            tc.tile_pool(name="sm", bufs=2) as smp,
            tc.tile_pool(name="psum", bufs=1, space="PSUM") as psp,
        ):
            xcs, x4s, x0cs = [], [], []
            d_xcs, d_x4s, d_x0cs = [], [], []
            for mg in range(N_MG):
                t = iop.tile([128, 512], f16, name=f"xc{mg}")
                d = nc.sync.dma_start(t[:], xc_d[mg])
                xcs.append(t); d_xcs.append(d)
                t = iop.tile([128, 512], f16, name=f"x0c{mg}")
                d = nc.sync.dma_start(t[:], x0c_d[mg])
                x0cs.append(t); d_x0cs.append(d)
            wn4 = constp.tile([128, 32], f16)
            d_wn4 = nc.sync.dma_start(wn4[:], wn4_d[:])
            aall = [constp.tile([128, 2048], f16, name="aall0")]
            d_aall0 = nc.sync.dma_start(aall[0][:], aall0_d[:])
            wnf2 = constp.tile([128, 2048], f16)
            d_wnf2 = nc.sync.dma_start(wnf2[:], wnf2_d[:])
            for mg in range(N_MG):
                t = iop.tile([128, 2048], f16, name=f"x4{mg}")
                d = nc.sync.dma_start(t[:], x4_d[mg])
                x4s.append(t); d_x4s.append(d)
            _alldmas += [d_wnf2, d_wn4, d_aall0] + d_xcs + d_x4s + d_x0cs

            ps = psp.tile([128, 4096], f32)   # R0=[0:2048], R1=[2048:4096]

            lasts = {"DVE": None, "Pool": None, "Act": None, "PE": None}

            def dve(fn, *a, **k):
                i = fn(*a, **k)
                lasts["DVE"] = i
                return i

            def pool(fn, *a, **k):
                i = fn(*a, **k)
                lasts["Pool"] = i
                return i

            def scal(out, in_, func, scale=1.0):
                i = act(out, in_, func, scale=scale)
                lasts["Act"] = i
                return i

            def scal2(out, in_, func, scale=1.0, bias=0.0):
                i = act(out, in_, func, scale=scale, bias=bias)
                lasts["Act"] = i
                return i

            # ============================================================
            # iteration 0 (pair-pipelined)
            # ============================================================
            grs32 = [bigp.tile([128, 2048], f16,
                               name=f"grs_{k}", tag="grs32")
                     for k in range(3)]
            gr_off0 = [2048 * (mg % 2) + 512 * (mg // 2) for mg in range(N_MG)]
            c0all = smp.tile([128, 256], f32, name="c0all")
            grad0_done, grs_dmas0 = {}, {}
            for pr in (0, 1):
                pmgs = (2 * pr, 2 * pr + 1)
                for mg in pmgs:
                    eng_fence(nc.gpsimd, [d_xcs[mg], d_x0cs[mg]])
                    m0 = bigp.tile([128, 512], f16, name=f"m0_{mg}",
                                   tag="m0")
                    i_m0 = pool(nc.gpsimd.tensor_tensor, m0[:], xcs[mg][:],
                                x0cs[mg][:], OP.mult)
                    eng_fence(nc.vector, [i_m0])
                    dve(nc.vector.tensor_reduce,
                        c0all[:, mg * 64:(mg + 1) * 64],
                        m0[:].rearrange("p (k d) -> p k d", d=8),
                        mybir.AxisListType.X, OP.add)

                sl0 = slice(128 * pr, 128 * pr + 128)
                t1_0 = smp.tile([128, 128], f16, name=f"t1_0{pr}")
                i_t10 = dve(nc.vector.tensor_scalar, t1_0[:], c0all[:, sl0],
                            0.999, -1.0, OP.mult, OP.add)
                u_0 = smp.tile([128, 128], f16, name=f"u_0{pr}")
                i_u0 = dve(nc.vector.scalar_tensor_tensor, u_0[:], t1_0[:],
                           2.0, t1_0[:], OP.add, OP.mult)
                rs_0 = smp.tile([128, 128], f16, name=f"rs_0{pr}")
                eng_fence(nc.scalar, [i_u0])
                i_rs0 = scal(rs_0[:], u_0[:], AF.Rsqrt, scale=-1.0)
                tn_0 = smp.tile([128, 128], f16, name=f"tn_0{pr}")
                eng_fence(nc.gpsimd, [i_rs0])
                i_tn0 = pool(nc.gpsimd.tensor_tensor, tn_0[:], t1_0[:],
                             rs_0[:], OP.mult)
                at_0 = smp.tile([128, 128], f16, name=f"at_0{pr}")
                eng_fence(nc.scalar, [i_tn0])
                i_at0 = scal(at_0[:], tn_0[:], AF.Arctan)
                q_0 = smp.tile([128, 128], f16, name=f"q_0{pr}")
                eng_fence(nc.vector, [i_at0])
                i_q0 = dve(nc.vector.scalar_tensor_tensor, q_0[:], at_0[:],
                           2.0, rs_0[:], OP.mult, OP.mult)

                eng_fence(nc.gpsimd, [i_q0])
                for mg in pmgs:
                    y = bigp.tile([128, 512], f16, name=f"y{mg}",
                                  tag=f"y{mg}")
                    qo = (mg - 2 * pr) * 64
                    qb = q_0[:, qo:qo + 64].rearrange(
                        "p (k o) -> p k o", o=1).broadcast_to((128, 64, 8))
                    i_y = pool(nc.gpsimd.tensor_tensor,
                               y[:].rearrange("p (k d) -> p k d", d=8),
                               xcs[mg][:].rearrange("p (k d) -> p k d", d=8),
                               qb, OP.mult)

                    fdeps = [i_y]
                    if mg == 0:
                        fdeps.append(d_wn4)
                    fence = pe_fence(fdeps)
                    lgm = None
                    for g in range(4):
                        for s in range(N_SG):
                            for j in range(4):
                                for p in range(4):
                                    _gm = nc.tensor.matmul(
                                        ps[32 * p:32 * p + 32,
                                           gr_off0[mg] + s * 128 + j * 32
                                           + g * 8:
                                           gr_off0[mg] + s * 128 + j * 32
                                           + g * 8 + 8],
                                        wn4[32 * g:32 * g + 32, :],
                                        y[32 * g:32 * g + 32,
                                          s * 128 + j * 32 + p * 8:
                                          s * 128 + j * 32 + p * 8 + 8],
                                        tile_position=(32 * g, 32 * p),
                                    )
                                    if fence is not None:
                                        add_dep_helper(_gm.ins, fence.ins,
                                                       sync=False,
                                                       reason="order")
                                    lgm = _gm
                    lasts["PE"] = lgm
                    grad0_done[mg] = lgm
                    dst = grs32[0][:, mg * 512:(mg + 1) * 512]
                    srcp = ps[:, gr_off0[mg]:gr_off0[mg] + 512]
                    eng_fence(nc.scalar, [lgm])
                    grs_dmas0[mg] = scal(dst, srcp, AF.Copy)
            grs_dmas0 = [grs_dmas0[mg] for mg in range(N_MG)]
            # region R[par] was used by it0 GRs of mgs with mg%2==par;
            # all it0 GRs sit inside half0 of each region
            prev_free = {}
            for mg in range(N_MG):
                om = (1, 3) if mg % 2 == 0 else (0, 2)
                prev_free[(mg, 0)] = [grs_dmas0[om[0]], grs_dmas0[om[1]]]
                prev_free[(mg, 1)] = []

            # ============================================================
            # generic exp-map (per pair), returns anew writes into aall[it+1]
            # ============================================================
            def exp_pair(it, pr, aold, anew, grsA, dep_dmas, fin32=False):
                """pr in (0,1): mgs (2pr, 2pr+1); slices [1024*pr : +1024]."""
                lo = 1024 * pr
                sl = slice(lo, lo + 1024)
                agg2 = bigp.tile([128, 2048], f16, name=f"agg2_{it}_{pr}",
                                 tag="agg2")
                eng_fence(nc.gpsimd, dep_dmas)
                i_ag = pool(nc.gpsimd.tensor_tensor, agg2[:, 0:1024],
                            aold[:, sl], grsA[:, sl], OP.mult)
                eng_fence(nc.scalar, dep_dmas)
                i_g2 = scal(agg2[:, 1024:2048], grsA[:, sl], AF.Square)
                gsg = smp.tile([128, 256], f32, name=f"gsg_{it}_{pr}",
                               tag="gsg")
                eng_fence(nc.vector, [i_ag])
                dve(nc.vector.tensor_reduce, gsg[:, 0:128],
                    agg2[:, 0:1024].rearrange("p (k d) -> p k d", d=8),
                    mybir.AxisListType.X, OP.add)
                eng_fence(nc.vector, [i_g2])
                dve(nc.vector.tensor_reduce, gsg[:, 128:256],
                    agg2[:, 1024:2048].rearrange("p (k d) -> p k d", d=8),
                    mybir.AxisListType.X, OP.add)
                gs = gsg[:, 0:128]
                sg2 = gsg[:, 128:256]
                gs2 = smp.tile([128, 128], f32, name=f"gs2_{it}_{pr}",
                               tag="gs2")
                dve(nc.vector.tensor_tensor, gs2[:], gs, gs, OP.mult)
                n2 = smp.tile([128, 128], f32, name=f"n2_{it}_{pr}", tag="n2")
                i_n2 = dve(nc.vector.tensor_tensor, n2[:], sg2, gs2[:],
                           OP.subtract)

                # Taylor sinc (deg3) / cos (deg4) in u = n2
                # sinc deg-2, cos deg-3 in u=n2 (n2 <= ~0.62)
                sc = smp.tile([128, 128], f32, name=f"sc_{it}_{pr}", tag="sc")
                tmp = smp.tile([128, 128], f32, name=f"tmp_{it}_{pr}",
                               tag="tmp")
                dve(nc.vector.tensor_scalar, tmp[:], n2[:], 1.0 / 120.0,
                    -1.0 / 6.0, OP.mult, OP.add)
                dve(nc.vector.scalar_tensor_tensor, sc[:], tmp[:], 0.0,
                    n2[:], OP.add, OP.mult)
                i_sc = dve(nc.vector.tensor_scalar, sc[:], sc[:], 1.0, None,
                           OP.add)

                cc = smp.tile([128, 128], f32, name=f"cc_{it}_{pr}", tag="cc")
                tmp2 = smp.tile([128, 128], f32, name=f"tmp2_{it}_{pr}",
                                tag="tmp2")
                dve(nc.vector.tensor_scalar, tmp2[:], n2[:], -1.0 / 720.0,
                    1.0 / 24.0, OP.mult, OP.add)
                dve(nc.vector.scalar_tensor_tensor, cc[:], tmp2[:], 0.0,
                    n2[:], OP.add, OP.mult)
                dve(nc.vector.tensor_scalar, tmp2[:], cc[:], -1.0 / 2.0, None,
                    OP.add)
                dve(nc.vector.scalar_tensor_tensor, cc[:], tmp2[:], 0.0,
                    n2[:], OP.add, OP.mult)
                i_cc = dve(nc.vector.tensor_scalar, cc[:], cc[:], 1.0, None,
                           OP.add)

                scgs = smp.tile([128, 128], f32, name=f"scgs_{it}_{pr}",
                                tag="scgs")
                eng_fence(nc.gpsimd, [i_sc])
                i_scgs = pool(nc.gpsimd.tensor_tensor, scgs[:], sc[:], gs,
                              OP.mult)
                ca = smp.tile([128, 128], f32, name=f"ca_{it}_{pr}", tag="ca")
                eng_fence(nc.vector, [i_cc, i_scgs])
                i_ca = dve(nc.vector.tensor_tensor, ca[:], cc[:], scgs[:],
                           OP.subtract)

                u1 = bigp.tile([128, 1024], f16, name=f"u1_{it}_{pr}",
                               tag="u1")
                scb = sc[:].rearrange("p (k o) -> p k o", o=1).broadcast_to(
                    (128, 128, 8))
                i_u1 = dve(nc.vector.tensor_tensor,
                           u1[:].rearrange("p (k d) -> p k d", d=8),
                           grsA[:, sl].rearrange("p (k d) -> p k d", d=8),
                           scb, OP.mult)
                u2 = bigp.tile([128, 1024], f16, name=f"u2_{it}_{pr}",
                               tag="u2")
                cab = ca[:].rearrange("p (k o) -> p k o", o=1).broadcast_to(
                    (128, 128, 8))
                i_u2 = dve(nc.vector.tensor_tensor,
                           u2[:].rearrange("p (k d) -> p k d", d=8),
                           aold[:, sl].rearrange("p (k d) -> p k d", d=8),
                           cab, OP.mult)
                i_an = dve(nc.vector.tensor_tensor, anew[:, sl], u1[:],
                           u2[:], OP.add)
                return i_an

            # transposes: tt(mg) from aall slice
            def make_tt(asrc, mg, dep):
                tt = bigp.tile([128, 512], f16, name=f"tt{mg}", tag="tt",
                               bufs=5)
                if dep is not None:
                    eng_fence(nc.vector, [dep])
                i_tt = dve(nc.vector.transpose, tt[:],
                           asrc[:, mg * 512:(mg + 1) * 512])
                return tt, i_tt

            # ============================================================
            # iterations: it0 exp, then (inner,chain,grad,exp) for it 1,2
            # ============================================================
            aall.append(constp.tile([128, 2048], f16, name="aall1"))
            aall.append(constp.tile([128, 2048], f16, name="aall2"))
            aout = constp.tile([128, 2048], f16, name="aout")

            # --- it0 exp pairs + transposes
            i_an00 = exp_pair(0, 0, aall[0], aall[1], grs32[0],
                              [d_aall0, grs_dmas0[0], grs_dmas0[1]])
            tt0, i_tt00 = make_tt(aall[1], 0, i_an00)
            tt1, i_tt01 = make_tt(aall[1], 1, None)
            i_an01 = exp_pair(0, 1, aall[0], aall[1], grs32[0],
                              [grs_dmas0[2], grs_dmas0[3]])
            tt2, i_tt02 = make_tt(aall[1], 2, i_an01)
            tt3, i_tt03 = make_tt(aall[1], 3, None)
            tts = [(tt0, i_tt00), (tt1, i_tt01), (tt2, i_tt02), (tt3, i_tt03)]

            for it in (1, 2):
                t1h, rsh, ath, fwh = {}, {}, {}, {}
                cd, innh, gradh = {}, {}, {}
                grs_dmas = []

                def inner_half(mg, h):
                    roff = 2048 * ((mg + it) % 2)
                    fdeps = []
                    if h == 0:
                        fdeps.append(tts[mg][1])
                        if it == 1:
                            fdeps.append(d_x4s[mg])
                        fdeps += prev_free[(mg, 0)]
                        if mg >= 2:
                            fdeps += cd[(mg - 2, 0)] + [grs_dmas[mg - 2]]
                    else:
                        fdeps += prev_free[(mg, 1)]
                        if mg >= 2:
                            fdeps += cd[(mg - 2, 1)]
                    fence = pe_fence(fdeps)
                    lim = None
                    for p in range(4):
                        for s in (2 * h, 2 * h + 1):
                            for j in range(4):
                                _mm = nc.tensor.matmul(
                                    ps[0:128,
                                       roff + s * 512 + j * 128 + p * 32:
                                       roff + s * 512 + j * 128 + p * 32 + 32],
                                    x4s[mg][32 * p:32 * p + 32,
                                            s * 512 + j * 128:
                                            s * 512 + j * 128 + 128],
                                    tts[mg][0][32 * p:32 * p + 32,
                                               s * 128 + j * 32:
                                               s * 128 + j * 32 + 32],
                                    tile_position=(32 * p, 0),
                                )
                                if fence is not None:
                                    add_dep_helper(_mm.ins, fence.ins,
                                                   sync=False, reason="order")
                                lim = _mm
                    lasts["PE"] = lim
                    innh[(mg, h)] = lim

                def extract_half(mg, h):
                    """t1 = 0.999*c - 1 (h0 DVE, h1 Act) for tn;
                    c2 = (0.999*c)^2 from PSUM on Act; rs = Rsqrt(1-c2)."""
                    roff = 2048 * ((mg + it) % 2) + 1024 * h
                    c2 = bigp.tile([128, 1024], f16, name=f"c2_{it}{mg}{h}",
                                   tag=f"c2{h}", bufs=3)
                    eng_fence(nc.scalar, [innh[(mg, h)]])
                    i_c2 = scal2(c2[:], ps[:, roff:roff + 1024],
                                 AF.Square, scale=0.999, bias=0.0)
                    t1 = bigp.tile([128, 1024], f16, name=f"t1_{it}{mg}{h}",
                                   tag=f"t1{h}", bufs=4)
                    if h == 0:
                        eng_fence(nc.vector, [innh[(mg, h)]])
                        i_t1 = dve(nc.vector.tensor_scalar, t1[:],
                                   ps[:, roff:roff + 1024], 0.999,
                                   -1.0, OP.mult, OP.add)
                    else:
                        i_t1 = scal2(t1[:], ps[:, roff:roff + 1024],
                                     AF.Copy, scale=0.999, bias=-1.0)
                    t1h[(mg, h)] = (t1, i_t1)
                    cd[(mg, h)] = [i_t1, i_c2]
                    return c2, i_c2, -1.0

                def grad_half(mg, h):
                    roff = 2048 * ((mg + it) % 2)
                    goff = roff
                    gdeps = [fwh[(mg, h)][1]]
                    if it == 1 and h == 0:
                        gdeps.append(d_xcs[mg])
                    if h == 1:
                        gdeps += cd[(mg, 0)]  # GR bank inside c-half0
                    if mg >= 2 and h == 0:
                        gdeps.append(grs_dmas[mg - 2])
                    fence = pe_fence(gdeps)
                    lgm = None
                    for g in range(4):
                        for s in (2 * h, 2 * h + 1):
                            for j in range(4):
                                for p in range(4):
                                    _gm = nc.tensor.matmul(
                                        ps[32 * p:32 * p + 32,
                                           goff + s * 128 + j * 32 + g * 8:
                                           goff + s * 128 + j * 32 + g * 8 + 8],
                                        fwh[(mg, h)][0][
                                            32 * g:32 * g + 32,
                                            (s - 2 * h) * 512 + j * 128
                                            + p * 32:
                                            (s - 2 * h) * 512 + j * 128
                                            + p * 32 + 32],
                                        xcs[mg][32 * g:32 * g + 32,
                                                s * 128 + j * 32 + p * 8:
                                                s * 128 + j * 32 + p * 8 + 8],
                                        tile_position=(32 * g, 32 * p),
                                    )
                                    if fence is not None:
                                        add_dep_helper(_gm.ins, fence.ins,
                                                       sync=False,
                                                       reason="order")
                                    lgm = _gm
                    lasts["PE"] = lgm
                    gradh[(mg, h)] = lgm

                for pr in (0, 1):
                    pmgs = (2 * pr, 2 * pr + 1)
                    us = {}
                    for mg in pmgs:
                        for h in (0, 1):
                            inner_half(mg, h)
                        for h in (0, 1):
                            us[(mg, h)] = extract_half(mg, h)
                    # rs batch (R table); h0: Rsqrt(-u), h1: Rsqrt(1-c2)
                    for mg in pmgs:
                        for h in (0, 1):
                            rs = bigp.tile([128, 1024], f16,
                                           name=f"rs_{it}{mg}{h}",
                                           tag=f"rs{h}", bufs=4)
                            i_rs = scal2(rs[:], us[(mg, h)][0][:],
                                         AF.Rsqrt, scale=-1.0, bias=1.0)
                            rsh[(mg, h)] = (rs, i_rs)
                    # tn on Pool
                    tns = {}
                    for mg in pmgs:
                        for h in (0, 1):
                            tn = bigp.tile([128, 1024], f16,
                                           name=f"tn_{it}{mg}{h}",
                                           tag=f"tn{h}")
                            eng_fence(nc.vector, [rsh[(mg, h)][1]])
                            i_tn = dve(nc.vector.tensor_tensor, tn[:],
                                       t1h[(mg, h)][0][:],
                                       rsh[(mg, h)][0][:], OP.mult)
                            tns[(mg, h)] = (tn, i_tn)
                    # at batch (T table)
                    for mg in pmgs:
                        for h in (0, 1):
                            at_ = bigp.tile([128, 1024], f16,
                                            name=f"at_{it}{mg}{h}",
                                            tag=f"at{h}", bufs=4)
                            eng_fence(nc.scalar, [tns[(mg, h)][1]])
                            i_at = scal(at_[:], tns[(mg, h)][0][:],
                                        AF.Arctan)
                            ath[(mg, h)] = (at_, i_at)
                    # rsW, fw, grad per half
                    for mg in pmgs:
                        for h in (0, 1):
                            woff = 1024 * h
                            rsW = bigp.tile([128, 1024], f16,
                                            name=f"rsW_{it}{mg}{h}",
                                            tag=f"rsW{h}")
                            if it == 1 and mg == 0 and h == 0:
                                eng_fence(nc.gpsimd, [d_wnf2])
                            eng_fence(nc.gpsimd, [rsh[(mg, h)][1]])
                            i_rsW = pool(nc.gpsimd.tensor_tensor, rsW[:],
                                         rsh[(mg, h)][0][:],
                                         wnf2[:, woff:woff + 1024], OP.mult)
                            fw = bigp.tile([128, 1024], f16,
                                           name=f"fw_{it}{mg}{h}",
                                           tag=f"fw{h}", bufs=4)
                            eng_fence(nc.vector, [ath[(mg, h)][1]])
                            i_fw = dve(nc.vector.tensor_tensor, fw[:],
                                       ath[(mg, h)][0][:], rsW[:], OP.mult)
                            fwh[(mg, h)] = (fw, i_fw)
                            grad_half(mg, h)
                        dst = grs32[it][:, mg * 512:(mg + 1) * 512]
                        goff = 2048 * ((mg + it) % 2)
                        eng_fence(nc.scalar, [gradh[(mg, 0)],
                                              gradh[(mg, 1)]])
                        dd = scal(dst, ps[:, goff:goff + 512], AF.Copy)
                        grs_dmas.append(dd)

                # next it region-free deps per (mg, h)
                prev_free = {}
                for mg in range(N_MG):
                    om = (1, 3) if mg % 2 == 0 else (0, 2)
                    prev_free[(mg, 0)] = ([grs_dmas[om[0]], grs_dmas[om[1]]]
                                          + cd[(om[0], 0)] + cd[(om[1], 0)])
                    prev_free[(mg, 1)] = cd[(om[0], 1)] + cd[(om[1], 1)]

                # exp pairs
                fin = (it == 2)
                dst = aout if fin else aall[it + 1]
                i_anA = exp_pair(it, 0, aall[it], dst, grs32[it],
                                 [grs_dmas[0], grs_dmas[1]], fin32=fin)
                if not fin:
                    ttA0, iA0 = make_tt(dst, 0, i_anA)
                    ttA1, iA1 = make_tt(dst, 1, None)
                i_anB = exp_pair(it, 1, aall[it], dst, grs32[it],
                                 [grs_dmas[2], grs_dmas[3]], fin32=fin)
                if not fin:
                    ttB2, iB2 = make_tt(dst, 2, i_anB)
                    ttB3, iB3 = make_tt(dst, 3, None)
                    tts = [(ttA0, iA0), (ttA1, iA1), (ttB2, iB2), (ttB3, iB3)]

            eng_fence(nc.sync, [i_anA])
            d_outA = nc.sync.dma_start(out_d[:, 0:1024], aout[:, 0:1024])
            eng_fence(nc.sync, [i_anB])
            d_outB = nc.sync.dma_start(out_d[:, 1024:2048],
                                       aout[:, 1024:2048])
            _alldmas += [d_outA, d_outB]

            eng_fence(nc.sync, _alldmas + [lasts["DVE"], lasts["Pool"],
                                           lasts["Act"], lasts["PE"],
                                           i_anA, i_anB])

    if strip:
        _strip_redundant_pe_waits(nc)
    return nc


def _strip_redundant_pe_waits(nc):
    """HW wait-slot limits: Matmult=1, engine ops=2 (use 1), DMA=2.  Drop
    (a) waits covered by an earlier same-engine wait, then (b) self-engine
    waits, when over the limit."""
    eng_sem = {"DVE": "DVE_", "Activation": "Activation_", "Pool": "Pool_",
               "PE": "PE_", "SP": "SP_"}
    covered = {}
    for inst in nc.all_instructions():
        eng = getattr(inst, "engine", None)
        ename = str(eng).split(".")[-1] if eng is not None else ""
        si = inst.sync_info
        if si is None:
            continue
        waits = list(si.on_wait or [])
        if not waits:
            continue
        tname = type(inst).__name__
        if tname in ("InstDrain", "InstEventSemaphore", "InstNoOp"):
            limit = 99
        elif tname == "InstMatmult":
            limit = 1
        else:
            limit = 1
        changed = False
        if len(waits) > limit and ename in eng_sem:
            nw = [x for x in waits
                  if covered.get((ename, x.id), -(10 ** 9)) < x.wait_value]
            if len(nw) != len(waits):
                waits, changed = nw, True
            pref = eng_sem.get(ename) if ename in ("DVE", "Activation",
                                                   "Pool") else None
            if len(waits) > limit and pref and tname != "InstDMACopy":
                nw = [x for x in waits if not x.ant_name.startswith(pref)]
                if len(nw) != len(waits):
                    waits, changed = nw, True
            if len(waits) > limit:
                raise RuntimeError(
                    f"{tname} {inst.name} ({ename}) has {len(waits)} "
                    f"uncovered waits: {[x.ant_name for x in waits]}")
        if changed:
            si.on_wait = waits
        for x in waits:
            if ename in eng_sem and covered.get((ename, x.id),
                                               -(10 ** 9)) < x.wait_value:
                covered[(ename, x.id)] = x.wait_value


# --------------------------------------------------------------------------
# entry point
# --------------------------------------------------------------------------
def kernel(x, w):
    from concourse.bass_utils import run_bass_kernel_spmd

    per_core = _host_prep(x, w)
    if "nc" not in _CACHE:
        _CACHE["nc"] = build_bass()
    nc = _CACHE["nc"]
    in_maps = [per_core[c] for c in range(NCORES)]
    res = run_bass_kernel_spmd(nc, in_maps, list(range(NCORES)))
    _CACHE["exec_time_ns"] = getattr(res, "exec_time_ns", None)
    outs = [res.results[c]["outa"] for c in range(NCORES)]
    return _host_unpack(outs)


if __name__ == "__main__":
    import sys
    sys.path.insert(0, "/root/problem")
    import reference

    inputs = reference.setup_inputs()
    out = kernel(**{k: np.asarray(v) for k, v in inputs.items()})
    print("kernel output shape:", out.shape, out.dtype)
